# revision 7
# baseline (speedup 1.0000x reference)
"""Low-pass FFT filtering kernel for Trainium2 (8 NeuronCores).

Math: reference does, per (batch b, channel i), with X = x[b,:,:,i] (256x256):
    out_i = irfft(rfft(X, axis=0) * mask) + irfft(rfft(X, axis=1) * mask)
with mask keeping rfft modes 0..15 (ortho norm). That filter is an orthogonal
projection P = W @ W.T where W [256, 31] is the orthonormal basis
{1/sqrt(n), sqrt(2/n)cos(2pi k t/n), -sqrt(2/n)sin(2pi k t/n)}_{k=1..15}.
So  out_i = P @ X_i + X_i @ P = W @ (W.T @ X_i) + (X_i @ W) @ W.T.

Device schedule (per core = one batch, channel-major layouts):
  C = W.T @ Xcm   [31, I*N]   (Xcm = x[b] as [m, (i, n)])
  D = W.T @ Xt    [31, I*M]   (Xt  = x[b] as [n, (i, m)], host-transposed)
  out[m-tile, n'] per (i, j):  single K=63 matmul with
     lhsT = [Wt_j ; 0 ; D_i,j]  (63 x 128),  rhs = [C_i ; 0 ; Wt] (63 x 256)
  which accumulates both terms in one PSUM pass.

v2 scheduling notes:
  - Lg/Rg are persistent SBUF buffers (3 each, rotated per chunk); their
    constant rows (tiled W^T and the zero row) are DMA'd once at startup
    instead of re-loaded from HBM per chunk.
  - One DMA per chunk per input stream using a [128, 2, w] pattern that
    covers both 128-row halves; xc rides the SP HWDGE ring, xt the ACT ring.
  - Outputs are issued per chunk on the GpSimd SWDGE ring (and the last
    chunk's halves on the SP ring tail) so they overlap the input stream
    instead of queueing behind it.
  - Phase-1 runs weight-major (all W-top matmuls of a chunk, then all
    W-bottom) to amortize LDWEIGHTS.
  - PSUM->SBUF traffic is spread over ACT (C rows), DVE (D rows) and
    ACT/DVE/Pool round-robin for the output casts.
Inputs/weights are fp16 on device; accumulation is fp32 in PSUM; output is
staged fp16 and upcast to fp32 on host (rel err ~7e-4 end to end).
Sharding: batch b -> core b (8 cores, no communication).
"""

import os
import sys
import types

import numpy as np

import concourse.bass as bass
import concourse.bacc as bacc
import concourse.tile as tile
from concourse import mybir
from concourse.bass_utils import run_bass_kernel_spmd

B, M, N, I = 8, 256, 256, 32
KMAX = 16           # modes kept: 0..15
R = 2 * KMAX - 1    # 31 real basis vectors
FREE = I * N        # 8192
CCOLS = 2048        # max chunk width (8 channels)
F32 = mybir.dt.float32
F16 = mybir.dt.float16
NPDT = np.float16

WIDTHS = [512, 1536, 2048, 2048, 2048]

LAST_RESULTS = None  # BassKernelResults of the most recent run (for test.py)


def _ensure_ntff_hook():
    """Provide antenv.axon_hooks if the image lacks it, so trace=True works."""
    try:
        from antenv.axon_hooks import get_axon_ntff_profile_hook  # noqa: F401
        return
    except ImportError:
        pass
    try:
        from trn_agent_boot.trn_boot import _ntff_profile_via_ctypes
        hook = _ntff_profile_via_ctypes("/opt/axon/libaxon_pjrt.so")
    except Exception:
        hook = None
    mod = types.ModuleType("antenv.axon_hooks")
    _state = {"hook": hook}
    mod.get_axon_ntff_profile_hook = lambda: _state["hook"]
    mod.set_axon_ntff_profile_hook = lambda h: _state.update(hook=h)
    sys.modules["antenv.axon_hooks"] = mod
    try:
        import antenv
        antenv.axon_hooks = mod
    except ImportError:
        pass


def _basis():
    t = np.arange(N)
    cols = [np.ones(N) / np.sqrt(N)]
    for k in range(1, KMAX):
        cols.append(np.sqrt(2.0 / N) * np.cos(2 * np.pi * k * t / N))
        cols.append(-np.sqrt(2.0 / N) * np.sin(2 * np.pi * k * t / N))
    return np.stack(cols, axis=1).astype(np.float32)  # [256, 31]


def _build_nc():
    nc = bacc.Bacc("TRN2", target_bir_lowering=False, debug=False,
                   enable_asserts=False, num_devices=8)

    xc = nc.declare_dram_parameter("xc", [M, FREE], F16, isOutput=False)
    xt = nc.declare_dram_parameter("xt", [N, I * M], F16, isOutput=False)
    w2 = nc.declare_dram_parameter("w2", [128, 2 * R], F16, isOutput=False)
    wz = nc.declare_dram_parameter("wz", [R + 1, CCOLS], F16, isOutput=False)
    zw = nc.declare_dram_parameter("zw", [R + 1, CCOLS], F16, isOutput=False)
    out = nc.declare_dram_parameter("out", [M, FREE], F16, isOutput=True)

    starts = [0]
    for w_ in WIDTHS[:-1]:
        starts.append(starts[-1] + w_)

    with tile.TileContext(nc) as tc:
        with (
            tc.tile_pool(name="const", bufs=1) as constp,
            tc.tile_pool(name="xin", bufs=len(WIDTHS)) as xin,
            tc.tile_pool(name="oput", bufs=len(WIDTHS)) as outp,
            tc.tile_pool(name="pcd", bufs=4, space=bass.MemorySpace.PSUM) as pcdp,
            tc.tile_pool(name="p2", bufs=4, space=bass.MemorySpace.PSUM) as p2p,
        ):
            w2sb = constp.tile([128, 2 * R], F16)
            nc.sync.dma_start(out=w2sb[:], in_=w2[:])

            # persistent L/R buffers; const rows loaded once. The first pair
            # rides the SP ring ahead of chunk-0 input so chunk-0 phase 2 is
            # never gated on them; the rest ride the GpSimd SWDGE ring.
            NLR = 3
            Lgs = [constp.tile([63, CCOLS], F16, name=f"Lg{k}")
                   for k in range(NLR)]
            Rgs = [constp.tile([63, CCOLS], F16, name=f"Rg{k}")
                   for k in range(NLR)]
            nc.sync.dma_start(out=Lgs[0][0:32, :], in_=wz[:])
            nc.sync.dma_start(out=Rgs[0][31:63, :], in_=zw[:])
            for k in range(1, NLR):
                nc.gpsimd.dma_start(out=Lgs[k][0:32, :], in_=wz[:])
                nc.gpsimd.dma_start(out=Rgs[k][31:63, :], in_=zw[:])

            def _cp_vector(o, i):
                nc.vector.tensor_copy(o, i)

            def _cp_scalar(o, i):
                nc.scalar.copy(o, i)

            # GPSIMD/Pool cannot read PSUM on TRN2: casts live on DVE + ACT
            cast_engines = [_cp_vector, _cp_scalar]
            cast_rr = 0

            for g, (c0, w) in enumerate(zip(starts, WIDTHS)):
                Lg = Lgs[g % NLR]
                Rg = Rgs[g % NLR]
                nf = w // 512

                # one DMA per stream per chunk covering both 128-row halves:
                # dim order [part(128), half(2), col(w)]
                xg = xin.tile([128, 2, w], F16, tag="x")
                tg = xin.tile([128, 2, w], F16, tag="t")
                src_x = xc[:, c0:c0 + w].rearrange("(h p) c -> p h c", h=2)
                src_t = xt[:, c0:c0 + w].rearrange("(h p) c -> p h c", h=2)
                nc.sync.dma_start(out=xg[:], in_=src_x)
                nc.scalar.dma_start(out=tg[:], in_=src_t)

                # phase 1, weight-major: W-top pass opens all f-tiles, W-bot
                # pass closes them; 4 LDWEIGHTS per chunk instead of 4 per
                # f-tile.
                pcs = []
                for f in range(nf):
                    fsl = slice(f * 512, (f + 1) * 512)
                    pc = pcdp.tile([63, 512], F32, tag="pcd")
                    pcs.append((pc, fsl))
                    nc.tensor.matmul(pc[0:R, :], w2sb[:, 0:R], xg[:, 0, fsl],
                                     start=True, stop=False)
                    nc.tensor.matmul(pc[32:63, :], w2sb[:, 0:R], tg[:, 0, fsl],
                                     start=True, stop=False)
                for f in range(nf):
                    pc, fsl = pcs[f]
                    nc.tensor.matmul(pc[0:R, :], w2sb[:, R:2 * R], xg[:, 1, fsl],
                                     start=False, stop=True)
                    nc.tensor.matmul(pc[32:63, :], w2sb[:, R:2 * R], tg[:, 1, fsl],
                                     start=False, stop=True)
                    # C rows on ACT, D rows on DVE
                    nc.scalar.copy(Rg[0:R, fsl], pc[0:R, :])
                    nc.vector.tensor_copy(Lg[32:63, fsl], pc[32:63, :])

                og = outp.tile([128, 2, w], F16, tag="o")
                # phase 2: two channels share one full PSUM bank -> one cast
                # per (pair, j); casts round-robin over DVE/ACT/Pool
                for ip in range(w // N // 2):
                    for j in range(2):
                        p2 = p2p.tile([128, 2 * N], F32, tag="p2")
                        for s in range(2):
                            il = 2 * ip + s
                            csl = slice(il * N, (il + 1) * N)
                            jsl = slice(il * N + j * 128, il * N + (j + 1) * 128)
                            nc.tensor.matmul(p2[:, s * N:(s + 1) * N],
                                             Lg[:, jsl], Rg[:, csl],
                                             start=True, stop=True)
                        eng = cast_engines[cast_rr % 2]
                        cast_rr += 1
                        eng(og[:, j, 2 * ip * N:(2 * ip + 2) * N], p2[:])

                dst = out[:, c0:c0 + w].rearrange("(h p) c -> p h c", h=2)
                if g == len(WIDTHS) - 1:
                    # last chunk: halves on the SP ring (inputs done by now)
                    h = w // 2
                    for hh in range(2):
                        dsth = out[:, c0 + hh * h:c0 + (hh + 1) * h].rearrange(
                            "(h p) c -> p h c", h=2)
                        nc.sync.dma_start(out=dsth,
                                          in_=og[:, :, hh * h:(hh + 1) * h])
                else:
                    nc.gpsimd.dma_start(out=dst, in_=og[:])

    nc.finalize()
    return nc


_NC = None


def kernel(x: np.ndarray) -> np.ndarray:
    global _NC, LAST_RESULTS
    x = np.asarray(x)
    assert x.shape == (B, M, N, I), x.shape

    W = _basis().astype(NPDT)          # [256, 31]
    Wt = W.T.copy()                    # [31, 256]
    w2_np = np.concatenate([W[0:128, :], W[128:256, :]], axis=1)  # [128, 62]
    wtile = np.tile(Wt, (1, CCOLS // N))                          # [31, 2048]
    wz_np = np.concatenate([wtile, np.zeros((1, CCOLS), NPDT)], axis=0)
    zw_np = np.concatenate([np.zeros((1, CCOLS), NPDT), wtile], axis=0)

    if _NC is None:
        _NC = _build_nc()

    xq = np.asarray(x, dtype=NPDT)
    in_maps = []
    for b in range(B):
        xcm = np.ascontiguousarray(xq[b].transpose(0, 2, 1)).reshape(M, FREE)
        xtm = np.ascontiguousarray(xq[b].transpose(1, 2, 0)).reshape(N, I * M)
        in_maps.append({
            "xc": xcm, "xt": xtm,
            "w2": w2_np, "wz": wz_np, "zw": zw_np,
        })

    trace = bool(int(os.environ.get("KERNEL_TRACE", "0")))
    if trace:
        _ensure_ntff_hook()
    last_err = None
    for attempt in range(3):
        try:
            LAST_RESULTS = run_bass_kernel_spmd(_NC, in_maps, list(range(B)),
                                                trace=trace and attempt == 0)
            break
        except Exception as e:  # rare transient NRT_EXEC_UNIT_UNRECOVERABLE
            last_err = e
            import time as _time
            _time.sleep(2.0)
            try:
                import jax
                jax.clear_caches()
                jax.extend.backend.clear_backends()
            except Exception:
                pass
    else:
        raise last_err

    out = np.empty((B, M, N, I), np.float32)
    for b in range(B):
        dev = LAST_RESULTS.results[b]["out"].astype(np.float32).reshape(M, I, N)
        out[b] = dev.transpose(0, 2, 1)
    return out


# revision 10
# speedup vs baseline: 1.1554x; 1.1554x over previous
"""Low-pass FFT filtering kernel for Trainium2 (8 NeuronCores).

Math: reference does, per (batch b, channel i), with X = x[b,:,:,i] (256x256):
    out_i = irfft(rfft(X, axis=0) * mask) + irfft(rfft(X, axis=1) * mask)
with mask keeping rfft modes 0..15 (ortho norm). That filter is an orthogonal
projection P = W @ W.T where W [256, 31] is the orthonormal basis
{1/sqrt(n), sqrt(2/n)cos(2pi k t/n), -sqrt(2/n)sin(2pi k t/n)}_{k=1..15}.
So  out_i = P @ X_i + X_i @ P = W @ (W.T @ X_i) + (X_i @ W) @ W.T.

Device schedule (per core = one batch, channel-major layouts):
  C = W.T @ Xcm   [31, I*N]   (Xcm = x[b] as [m, (i, n)])
  D = W.T @ Xt    [31, I*M]   (Xt  = x[b] as [n, (i, m)], host-transposed)
  out[m-tile, n'] per (i, j):  single K=63 matmul with
     lhsT = [Wt_j ; 0 ; D_i,j]  (63 x 128),  rhs = [C_i ; 0 ; Wt] (63 x 256)
  which accumulates both terms in one PSUM pass.

v2 scheduling notes:
  - Lg/Rg are persistent SBUF buffers (3 each, rotated per chunk); their
    constant rows (tiled W^T and the zero row) are DMA'd once at startup
    instead of re-loaded from HBM per chunk.
  - One DMA per chunk per input stream using a [128, 2, w] pattern that
    covers both 128-row halves; xc rides the SP HWDGE ring, xt the ACT ring.
  - Outputs are issued per chunk on the GpSimd SWDGE ring (and the last
    chunk's halves on the SP ring tail) so they overlap the input stream
    instead of queueing behind it.
  - Phase-1 runs weight-major (all W-top matmuls of a chunk, then all
    W-bottom) to amortize LDWEIGHTS.
  - PSUM->SBUF traffic is spread over ACT (C rows), DVE (D rows) and
    ACT/DVE/Pool round-robin for the output casts.
Inputs/weights are fp16 on device; accumulation is fp32 in PSUM; output is
staged fp16 and upcast to fp32 on host (rel err ~7e-4 end to end).
Sharding: batch b -> core b (8 cores, no communication).
"""

import os
import sys
import types

import numpy as np

import concourse.bass as bass
import concourse.bacc as bacc
import concourse.tile as tile
from concourse import mybir
from concourse.bass_utils import run_bass_kernel_spmd

B, M, N, I = 8, 256, 256, 32
KMAX = 16           # modes kept: 0..15
R = 2 * KMAX - 1    # 31 real basis vectors
FREE = I * N        # 8192
CCOLS = 2048        # max chunk width (8 channels)
F32 = mybir.dt.float32
F16 = mybir.dt.float16
NPDT = np.float16

WIDTHS = [512, 1536, 2048, 2048, 2048]

LAST_RESULTS = None  # BassKernelResults of the most recent run (for test.py)


def _ensure_ntff_hook():
    """Provide antenv.axon_hooks if the image lacks it, so trace=True works."""
    try:
        from antenv.axon_hooks import get_axon_ntff_profile_hook  # noqa: F401
        return
    except ImportError:
        pass
    try:
        from trn_agent_boot.trn_boot import _ntff_profile_via_ctypes
        hook = _ntff_profile_via_ctypes("/opt/axon/libaxon_pjrt.so")
    except Exception:
        hook = None
    mod = types.ModuleType("antenv.axon_hooks")
    _state = {"hook": hook}
    mod.get_axon_ntff_profile_hook = lambda: _state["hook"]
    mod.set_axon_ntff_profile_hook = lambda h: _state.update(hook=h)
    sys.modules["antenv.axon_hooks"] = mod
    try:
        import antenv
        antenv.axon_hooks = mod
    except ImportError:
        pass


def _basis():
    t = np.arange(N)
    cols = [np.ones(N) / np.sqrt(N)]
    for k in range(1, KMAX):
        cols.append(np.sqrt(2.0 / N) * np.cos(2 * np.pi * k * t / N))
        cols.append(-np.sqrt(2.0 / N) * np.sin(2 * np.pi * k * t / N))
    return np.stack(cols, axis=1).astype(np.float32)  # [256, 31]


def _build_nc():
    nc = bacc.Bacc("TRN2", target_bir_lowering=False, debug=False,
                   enable_asserts=False, num_devices=8)

    xc = nc.declare_dram_parameter("xc", [M, FREE], F16, isOutput=False)
    xt = nc.declare_dram_parameter("xt", [N, I * M], F16, isOutput=False)
    w2 = nc.declare_dram_parameter("w2", [128, 2 * R], F16, isOutput=False)
    wz = nc.declare_dram_parameter("wz", [R + 1, CCOLS], F16, isOutput=False)
    zw = nc.declare_dram_parameter("zw", [R + 1, CCOLS], F16, isOutput=False)
    out = nc.declare_dram_parameter("out", [M, FREE], F16, isOutput=True)

    starts = [0]
    for w_ in WIDTHS[:-1]:
        starts.append(starts[-1] + w_)

    with tile.TileContext(nc) as tc:
        with (
            tc.tile_pool(name="const", bufs=1) as constp,
            tc.tile_pool(name="xin", bufs=len(WIDTHS)) as xin,
            tc.tile_pool(name="oput", bufs=len(WIDTHS)) as outp,
            tc.tile_pool(name="pcd", bufs=4, space=bass.MemorySpace.PSUM) as pcdp,
            tc.tile_pool(name="p2", bufs=4, space=bass.MemorySpace.PSUM) as p2p,
        ):
            w2sb = constp.tile([128, 2 * R], F16)
            nc.sync.dma_start(out=w2sb[:], in_=w2[:])

            NLR = 3
            Lgs = [constp.tile([63, CCOLS], F16, name=f"Lg{k}")
                   for k in range(NLR)]
            Rgs = [constp.tile([63, CCOLS], F16, name=f"Rg{k}")
                   for k in range(NLR)]

            # all input DMAs are issued up front so the SP/ACT sequencers
            # never block input descriptor-gen behind an output that is
            # waiting on casts. Chunk 0 (x and t) rides SP first so the PE
            # starts ASAP; outputs go on the SP ring tail, FIFO behind all
            # inputs = strict input priority on HBM bandwidth.
            xgs, tgs = [], []
            for g, (c0, w) in enumerate(zip(starts, WIDTHS)):
                xg = xin.tile([128, 2, w], F16, tag="x", name=f"xg{g}")
                tg = xin.tile([128, 2, w], F16, tag="t", name=f"tg{g}")
                xgs.append(xg)
                tgs.append(tg)
            nc.sync.dma_start(
                out=xgs[0][:],
                in_=xc[:, 0:WIDTHS[0]].rearrange("(h p) c -> p h c", h=2))
            nc.sync.dma_start(
                out=tgs[0][:],
                in_=xt[:, 0:WIDTHS[0]].rearrange("(h p) c -> p h c", h=2))
            # first L/R const pair on ACT (small, early, ahead of tg1..4)
            nc.scalar.dma_start(out=Lgs[0][0:32, :], in_=wz[:])
            nc.scalar.dma_start(out=Rgs[0][31:63, :], in_=zw[:])
            for g in range(1, len(WIDTHS)):
                c0, w = starts[g], WIDTHS[g]
                nc.sync.dma_start(
                    out=xgs[g][:],
                    in_=xc[:, c0:c0 + w].rearrange("(h p) c -> p h c", h=2))
            for g in range(1, len(WIDTHS)):
                c0, w = starts[g], WIDTHS[g]
                nc.scalar.dma_start(
                    out=tgs[g][:],
                    in_=xt[:, c0:c0 + w].rearrange("(h p) c -> p h c", h=2))
            # remaining const pairs on the (otherwise idle) GpSimd SWDGE ring
            for k in range(1, NLR):
                nc.gpsimd.dma_start(out=Lgs[k][0:32, :], in_=wz[:])
                nc.gpsimd.dma_start(out=Rgs[k][31:63, :], in_=zw[:])

            def _cp_vector(o, i):
                nc.vector.tensor_copy(o, i)

            def _cp_scalar(o, i):
                nc.scalar.copy(o, i)

            # GPSIMD/Pool cannot read PSUM on TRN2: casts live on DVE + ACT
            cast_engines = [_cp_vector, _cp_scalar]
            cast_rr = 0

            for g, (c0, w) in enumerate(zip(starts, WIDTHS)):
                Lg = Lgs[g % NLR]
                Rg = Rgs[g % NLR]
                xg = xgs[g]
                tg = tgs[g]
                nf = w // 512

                # phase 1, weight-major: W-top pass opens all f-tiles, W-bot
                # pass closes them; 4 LDWEIGHTS per chunk instead of 4 per
                # f-tile.
                pcs = []
                for f in range(nf):
                    fsl = slice(f * 512, (f + 1) * 512)
                    pc = pcdp.tile([63, 512], F32, tag="pcd")
                    pcs.append((pc, fsl))
                    nc.tensor.matmul(pc[0:R, :], w2sb[:, 0:R], xg[:, 0, fsl],
                                     start=True, stop=False)
                    nc.tensor.matmul(pc[32:63, :], w2sb[:, 0:R], tg[:, 0, fsl],
                                     start=True, stop=False)
                for f in range(nf):
                    pc, fsl = pcs[f]
                    nc.tensor.matmul(pc[0:R, :], w2sb[:, R:2 * R], xg[:, 1, fsl],
                                     start=False, stop=True)
                    nc.tensor.matmul(pc[32:63, :], w2sb[:, R:2 * R], tg[:, 1, fsl],
                                     start=False, stop=True)
                    # C rows on ACT, D rows on DVE
                    nc.scalar.copy(Rg[0:R, fsl], pc[0:R, :])
                    nc.vector.tensor_copy(Lg[32:63, fsl], pc[32:63, :])

                og = outp.tile([128, 2, w], F16, tag="o")
                # phase 2: two channels share one full PSUM bank -> one cast
                # per (pair, j); casts round-robin over DVE/ACT/Pool
                for ip in range(w // N // 2):
                    for j in range(2):
                        p2 = p2p.tile([128, 2 * N], F32, tag="p2")
                        for s in range(2):
                            il = 2 * ip + s
                            csl = slice(il * N, (il + 1) * N)
                            jsl = slice(il * N + j * 128, il * N + (j + 1) * 128)
                            nc.tensor.matmul(p2[:, s * N:(s + 1) * N],
                                             Lg[:, jsl], Rg[:, csl],
                                             start=True, stop=True)
                        eng = cast_engines[cast_rr % 2]
                        cast_rr += 1
                        eng(og[:, j, 2 * ip * N:(2 * ip + 2) * N], p2[:])

                if g == len(WIDTHS) - 1:
                    # last chunk ships in halves so the first half overlaps
                    # the final casts
                    h = w // 2
                    for hh in range(2):
                        dsth = out[:, c0 + hh * h:c0 + (hh + 1) * h].rearrange(
                            "(h p) c -> p h c", h=2)
                        nc.sync.dma_start(out=dsth,
                                          in_=og[:, :, hh * h:(hh + 1) * h])
                else:
                    dst = out[:, c0:c0 + w].rearrange("(h p) c -> p h c", h=2)
                    nc.sync.dma_start(out=dst, in_=og[:])

    nc.finalize()
    return nc


_NC = None


def kernel(x: np.ndarray) -> np.ndarray:
    global _NC, LAST_RESULTS
    x = np.asarray(x)
    assert x.shape == (B, M, N, I), x.shape

    W = _basis().astype(NPDT)          # [256, 31]
    Wt = W.T.copy()                    # [31, 256]
    w2_np = np.concatenate([W[0:128, :], W[128:256, :]], axis=1)  # [128, 62]
    wtile = np.tile(Wt, (1, CCOLS // N))                          # [31, 2048]
    wz_np = np.concatenate([wtile, np.zeros((1, CCOLS), NPDT)], axis=0)
    zw_np = np.concatenate([np.zeros((1, CCOLS), NPDT), wtile], axis=0)

    if _NC is None:
        _NC = _build_nc()

    xq = np.asarray(x, dtype=NPDT)
    in_maps = []
    for b in range(B):
        xcm = np.ascontiguousarray(xq[b].transpose(0, 2, 1)).reshape(M, FREE)
        xtm = np.ascontiguousarray(xq[b].transpose(1, 2, 0)).reshape(N, I * M)
        in_maps.append({
            "xc": xcm, "xt": xtm,
            "w2": w2_np, "wz": wz_np, "zw": zw_np,
        })

    trace = bool(int(os.environ.get("KERNEL_TRACE", "0")))
    if trace:
        _ensure_ntff_hook()
    last_err = None
    for attempt in range(3):
        try:
            LAST_RESULTS = run_bass_kernel_spmd(_NC, in_maps, list(range(B)),
                                                trace=trace and attempt == 0)
            break
        except Exception as e:  # rare transient NRT_EXEC_UNIT_UNRECOVERABLE
            last_err = e
            import time as _time
            _time.sleep(2.0)
            try:
                import jax
                jax.clear_caches()
                jax.extend.backend.clear_backends()
            except Exception:
                pass
    else:
        raise last_err

    out = np.empty((B, M, N, I), np.float32)
    for b in range(B):
        dev = LAST_RESULTS.results[b]["out"].astype(np.float32).reshape(M, I, N)
        out[b] = dev.transpose(0, 2, 1)
    return out
